# revision 6
# baseline (speedup 1.0000x reference)
"""GQA attention (B=1, S=2048, 32 Q / 8 KV heads, RoPE, causal) on 8 trn2
cores, head-parallel.  v2: single software-pipelined pass —
  iter0:  KV projections for all chunks (+Q chunk 0), V produced
          pre-transposed (stationary=xt, moving=wv)
  iter qc (1..3): Q projection of chunk qc interleaved with attention
          units of chunk qc-1; softmax epilogues deferred to iteration end
  flush:  attention chunk 3 interleaved with first half of the wo
          projection, then the rest of wo.
bf16 weights/activations (PSUM f32), bf16 partial-output store, host sum."""

import sys

if "/opt/trn_rl_repo" not in sys.path:
    sys.path.insert(0, "/opt/trn_rl_repo")

import contextlib

import numpy as np

import concourse.bacc as bacc
import concourse.mybir as mybir
import concourse.tile as tile
from concourse.bass_utils import run_bass_kernel_spmd

F32 = mybir.dt.float32
F32R = mybir.dt.float32r
BF16 = mybir.dt.bfloat16
EXP = mybir.ActivationFunctionType.Exp

HIDDEN = 4096
S = 2048
HD = 128
NCORES = 8
QH = 4
KT_H = HIDDEN // 128
NQC = S // 512
NST = S // 128

_CACHE = {}


def _interleave(fill, nslots):
    """Split list `fill` into nslots chunks, early slots first."""
    out = []
    n = len(fill)
    for i in range(nslots):
        out.append(fill[(i * n) // nslots:((i + 1) * n) // nslots])
    return out


def _build(reps=1, loop_n=0):
    nc = bacc.Bacc(None, target_bir_lowering=False)

    xt = nc.dram_tensor("xt", [HIDDEN, S], BF16, kind="ExternalInput")
    wq = nc.dram_tensor("wq", [HIDDEN, QH * HD], BF16, kind="ExternalInput")
    wk = nc.dram_tensor("wk", [HIDDEN, HD], BF16, kind="ExternalInput")
    wv = nc.dram_tensor("wv", [HIDDEN, HD], BF16, kind="ExternalInput")
    wo = nc.dram_tensor("wo", [QH * HD, HIDDEN], BF16, kind="ExternalInput")
    cosd = nc.dram_tensor("cosd", [64, S], F32, kind="ExternalInput")
    sind = nc.dram_tensor("sind", [64, S], F32, kind="ExternalInput")
    maskd = [nc.dram_tensor(f"mask{o}", [128, 512], BF16, kind="ExternalInput")
             for o in range(4)]
    onesc = nc.dram_tensor("onesc", [128, 1], BF16, kind="ExternalInput")
    onesr = nc.dram_tensor("onesr", [1, 128], F32, kind="ExternalInput")
    out = nc.dram_tensor("out", [HIDDEN, S], BF16, kind="ExternalOutput")

    with tile.TileContext(nc) as tc:
      with (tc.For_i(0, loop_n, 1) if loop_n else contextlib.nullcontext()):
       for _rep in range(reps):
        with tc.tile_pool(name="persist", bufs=1) as persist, \
             tc.tile_pool(name="rope", bufs=1) as p_rope, \
             tc.tile_pool(name="xtp", bufs=4) as p_xt, \
             tc.tile_pool(name="pp", bufs=4) as p_p, \
             tc.tile_pool(name="dacc", bufs=8) as p_dacc, \
             tc.tile_pool(name="aun", bufs=8) as p_aun, \
             tc.tile_pool(name="recip", bufs=2) as p_recip, \
             tc.tile_pool(name="rbc", bufs=2) as p_r, \
             tc.tile_pool(name="obp", bufs=4) as p_ob:
            qt = {(h, qc): persist.tile([128, 512], BF16, tag=f"qt{h}_{qc}",
                                        name=f"qt{h}_{qc}")
                  for h in range(QH) for qc in range(NQC)}
            kt = [persist.tile([128, 512], BF16, tag=f"kt{qc}", name=f"kt{qc}")
                  for qc in range(NQC)]
            v_sb = [persist.tile([128, 128], BF16, tag=f"v{j}", name=f"v{j}")
                    for j in range(NST)]
            aot = {(h, qc): persist.tile([128, 512], BF16, tag=f"ao{h}_{qc}",
                                         name=f"ao{h}_{qc}")
                   for h in range(QH) for qc in range(NQC)}
            cs_sb = persist.tile([128, S], F32, tag="cs", name="cs")
            snc_sb = persist.tile([128, S], F32, tag="snc", name="snc")
            mask_sb = [persist.tile([128, 512], BF16, tag=f"mask{o}",
                                    name=f"mask{o}") for o in range(4)]
            onesc_sb = persist.tile([128, 1], BF16, tag="onesc", name="onesc")
            onesr_sb = persist.tile([1, 128], F32R, tag="onesr", name="onesr")
            wq_t = [persist.tile([128, 4, 512], BF16, tag=f"wq8_{k4}",
                                 name=f"wq8_{k4}") for k4 in range(KT_H // 4)]
            wk_t = [persist.tile([128, 128], BF16, tag=f"wk{k}", name=f"wk{k}")
                    for k in range(KT_H)]
            wv_t = [persist.tile([128, 128], BF16, tag=f"wv{k}", name=f"wv{k}")
                    for k in range(KT_H)]
            wo_t = {}
            for c in range(4):
                for n4 in range(8):
                    wo_t[(c, n4)] = persist.tile(
                        [128, 512], BF16, tag=f"wo{c}_{n4}", name=f"wo{c}_{n4}")

            # constant loads spread over queues; first-needed tiles first:
            # wq on scalar, wk/wv on gpsimd (parallel queues), then tables,
            # then wo (needed only at the flush).
            for k4 in range(KT_H // 4):
                nc.scalar.dma_start(
                    out=wq_t[k4],
                    in_=wq[k4 * 512:(k4 + 1) * 512, :]
                        .rearrange("(j p) c -> p j c", p=128))
            for k in range(KT_H):
                nc.gpsimd.dma_start(out=wk_t[k], in_=wk[k * 128:(k + 1) * 128, :])
                nc.gpsimd.dma_start(out=wv_t[k], in_=wv[k * 128:(k + 1) * 128, :])
            nc.gpsimd.dma_start(out=cs_sb[0:64, :], in_=cosd[:, :])
            nc.gpsimd.dma_start(out=cs_sb[64:128, :], in_=sind[:, :])
            nc.gpsimd.dma_start(out=snc_sb[0:64, :], in_=sind[:, :])
            nc.gpsimd.dma_start(out=snc_sb[64:128, :], in_=cosd[:, :])
            for o in range(4):
                nc.gpsimd.dma_start(out=mask_sb[o], in_=maskd[o][:, :])
            nc.gpsimd.dma_start(out=onesc_sb, in_=onesc[:, :])
            nc.gpsimd.dma_start(out=onesr_sb, in_=onesr[:, :].bitcast(F32R))
            for c in range(4):
                for n4 in range(8):
                    nc.scalar.dma_start(
                        out=wo_t[(c, n4)],
                        in_=wo[c * 128:(c + 1) * 128, n4 * 512:(n4 + 1) * 512])

            def rope(src, dst, qc):
                # src: PSUM [128,512] f32 (deinterleaved head-dim: evens in
                # partitions 0:64, odds 64:128); DVE multiplies read PSUM
                # directly, cross-half combine on gpsimd.
                qs = slice(qc * 512, (qc + 1) * 512)
                ec = p_rope.tile([64, 512], F32, tag="ec", name="ec")
                os_ = p_rope.tile([64, 512], F32, tag="os", name="os_")
                es = p_rope.tile([64, 512], F32, tag="es", name="es")
                oc = p_rope.tile([64, 512], F32, tag="oc", name="oc")
                nc.vector.tensor_mul(ec, src[0:64, :], cs_sb[0:64, qs])
                nc.vector.tensor_mul(os_, src[64:128, :], cs_sb[64:128, qs])
                nc.vector.tensor_mul(es, src[0:64, :], snc_sb[0:64, qs])
                nc.vector.tensor_mul(oc, src[64:128, :], snc_sb[64:128, qs])
                nc.gpsimd.tensor_sub(dst[0:64, :], ec, os_)
                nc.gpsimd.tensor_add(dst[64:128, :], es, oc)

            with tc.tile_pool(name="psq", bufs=1, space="PSUM") as q_pool:

                def load_xt(qc, k2):
                    t = p_xt.tile([128, 2, 512], BF16, tag="xt", name="xt4")
                    nc.sync.dma_start(
                        out=t,
                        in_=xt[k2 * 256:(k2 + 1) * 256,
                               qc * 512:(qc + 1) * 512]
                            .rearrange("(j p) c -> p j c", p=128))
                    return t

                # ---------------- iter0: Q(0) + all KV -------------------
                with tc.tile_pool(name="pskv", bufs=2, space="PSUM") as kv_pool:
                    for qc in range(NQC):
                        psq = None
                        if qc == 0:
                            psq = [q_pool.tile([128, 512], F32, tag=f"q{d}",
                                               name=f"psq{d}")
                                   for d in range(QH)]
                        psk = kv_pool.tile([128, 512], F32, tag="k", name="psk")
                        psvT = kv_pool.tile([128, 512], F32, tag="v", name="psvT")
                        xt4 = None
                        for k in range(KT_H):
                            if k % 2 == 0:
                                xt4 = load_xt(qc, k // 2)
                            xt_t = xt4[:, k % 2, :]
                            st, sp = (k == 0), (k == KT_H - 1)
                            if qc == 0:
                                for d in range(QH):
                                    nc.tensor.matmul(
                                        psq[d][:, :],
                                        wq_t[k // 4][:, k % 4,
                                                     d * 128:(d + 1) * 128],
                                        xt_t, start=st, stop=sp)
                            nc.tensor.matmul(psk[:, :], wk_t[k][:, :], xt_t,
                                             start=st, stop=sp)
                            # V pre-transposed: out [keys, hd].  Four
                            # accumulation chains share one PSUM bank; the
                            # start bit zeroes the whole 2KB zero-region, so
                            # only chain 0 starts and only chain 3 stops.
                            for j in range(4):
                                nc.tensor.matmul(
                                    psvT[:, j * 128:(j + 1) * 128],
                                    xt_t[:, j * 128:(j + 1) * 128],
                                    wv_t[k][:, :],
                                    start=(st and j == 0),
                                    stop=(sp and j == 3))
                        rope(psk, kt[qc], qc)
                        for j in range(4):
                            nc.scalar.copy(v_sb[qc * 4 + j][:, :],
                                           psvT[:, j * 128:(j + 1) * 128])
                        if qc == 0:
                            for d in range(QH):
                                rope(psq[d], qt[(d, 0)], 0)

                # ---------------- iters 1..3 + flush ----------------------
                with tc.tile_pool(name="ps2s", bufs=1, space="PSUM") as p2s, \
                     tc.tile_pool(name="ps2o", bufs=2, space="PSUM") as p2o:
                    kstate = {}

                    def _off(qc, g, u):
                        # for diagonal key-block o = j-4qc, queries < o*128
                        # are fully masked — skip those columns.  g == 0 must
                        # stay full-width: its masked exp seeds every dacc
                        # column via the tensor_copy.
                        if g == 0:
                            return 0
                        o = 2 * g + u - 4 * qc
                        return max(0, o) * 128

                    def produce(h, qc, g):
                        if g == 0:
                            kstate[(h, qc)] = [
                                p2o.tile([128, 512], F32, tag="o", name="ps_o"),
                                p_dacc.tile([128, 1024], BF16, tag="dacc",
                                            name="dacc"),
                                {},
                            ]
                        ps_o, dacc, handles = kstate[(h, qc)]
                        ps_s = p2s.tile([128, 1024], F32, tag="s", name="ps_s")
                        for u in range(2):
                            j = 2 * g + u
                            off = _off(qc, g, u)
                            nc.tensor.matmul(
                                ps_s[:, u * 512 + off:(u + 1) * 512],
                                kt[j // 4][:, (j % 4) * 128:(j % 4 + 1) * 128],
                                qt[(h, qc)][:, off:512],
                                start=True, stop=True)
                        handles[("s", g)] = ps_s

                    def expmask(h, qc, g):
                        ps_o, dacc, handles = kstate[(h, qc)]
                        ps_s = handles.pop(("s", g))
                        p_t = p_p.tile([128, 1024], BF16, tag="p", name="p_t")
                        offs = [_off(qc, g, u) for u in range(2)]
                        if offs[0] == offs[1] == 0:
                            nc.scalar.activation(p_t[:, :], ps_s[:, :], EXP)
                        else:
                            for u in range(2):
                                sl = slice(u * 512 + offs[u], (u + 1) * 512)
                                nc.scalar.activation(p_t[:, sl], ps_s[:, sl],
                                                     EXP)
                        for u in range(2):
                            o = 2 * g + u - 4 * qc
                            if o >= 0:
                                off = offs[u]
                                sl = slice(u * 512 + off, (u + 1) * 512)
                                nc.vector.tensor_mul(
                                    p_t[:, sl], p_t[:, sl],
                                    mask_sb[o][:, off:512])
                        if g == 0:
                            nc.vector.tensor_copy(dacc[:, :], p_t[:, :])
                        elif offs[0] == offs[1] == 0:
                            nc.vector.tensor_add(dacc[:, :], dacc[:, :],
                                                 p_t[:, :])
                        else:
                            for u in range(2):
                                sl = slice(u * 512 + offs[u], (u + 1) * 512)
                                nc.vector.tensor_add(dacc[:, sl], dacc[:, sl],
                                                     p_t[:, sl])
                        handles[("p", g)] = p_t

                    def pv(h, qc, g, n_kt):
                        ps_o, dacc, handles = kstate[(h, qc)]
                        p_t = handles.pop(("p", g))
                        for u in range(2):
                            j = 2 * g + u
                            off = _off(qc, g, u)
                            nc.tensor.matmul(
                                ps_o[:, off:512], v_sb[j][:, :],
                                p_t[:, u * 512 + off:(u + 1) * 512],
                                start=(j == 0),
                                stop=(j == n_kt - 1))
                        if j == n_kt - 1:
                            aun = p_aun.tile([128, 512], BF16, tag="aun",
                                             name="aot_un")
                            nc.scalar.copy(aun[:, :], ps_o[:, :])
                            kstate[(h, qc)].append(aun)

                    def epilogues(qc):
                        st = [kstate.pop((h, qc)) for h in range(QH)]
                        psd, rc = [], []
                        for h in range(QH):
                            dacc = st[h][1]
                            ps_d = p2o.tile([128, 512], F32, tag="o",
                                            name="ps_d")
                            nc.tensor.matmul(ps_d[0:1, :], onesc_sb[:, :],
                                             dacc[:, 0:512],
                                             start=True, stop=False)
                            nc.tensor.matmul(ps_d[0:1, :], onesc_sb[:, :],
                                             dacc[:, 512:1024],
                                             start=False, stop=True)
                            psd.append(ps_d)
                        for h in range(QH):
                            recip = p_recip.tile([1, 512], F32R, tag="rc",
                                                 name="recip")
                            with nc.allow_low_precision(
                                    reason="softmax denom reciprocal"):
                                nc.vector.reciprocal(recip[:, :],
                                                     psd[h][0:1, :])
                            rc.append(recip)
                        for h in range(QH):
                            ps_rf = p2s.tile([128, 1024], F32, tag="s",
                                             name="ps_rf")
                            nc.tensor.matmul(ps_rf[:, 0:512], onesr_sb[:, :],
                                             rc[h][:, :], start=True, stop=True)
                            r_sb = p_r.tile([128, 512], F32, tag="r",
                                            name="r_sb")
                            nc.scalar.copy(r_sb[:, :], ps_rf[:, 0:512])
                            nc.vector.tensor_mul(aot[(h, qc)][:, :],
                                                 st[h][3][:, :], r_sb[:, :])

                    def p3_chain(ci, rt, u, dve_drain=False):
                        ps = q_pool.tile([128, 512], F32, tag=f"q{ci % 4}",
                                         name="ps3")
                        for c in range(4):
                            nc.tensor.matmul(
                                ps[:, :],
                                wo_t[(c, rt // 4)][:, (rt % 4) * 128:
                                                   (rt % 4 + 1) * 128],
                                aot[(c, u)][:, :],
                                start=(c == 0), stop=(c == 3))
                        ob = p_ob.tile([128, 512], BF16, tag="ob", name="ob")
                        if dve_drain and ci % 2 == 0:
                            # flush: ACT is busy with exp(); drain half on DVE
                            nc.vector.tensor_copy(ob[:, :], ps[:, :])
                        else:
                            nc.scalar.copy(ob[:, :], ps[:, :])
                        nc.sync.dma_start(
                            out=out[rt * 128:(rt + 1) * 128,
                                    u * 512:(u + 1) * 512],
                            in_=ob[:, :])

                    def attn_iter(qca, fill):
                        n_g = 2 * (qca + 1)
                        units = [(h, g) for h in range(QH) for g in range(n_g)]
                        nu = len(units)
                        # concentrate fill in the first nu slots so its tail
                        # (RoPE drains) lands ahead of the last units' DVE work
                        slots = _interleave(fill, nu) + [[], [], [], []]
                        for idx in range(nu + 4):
                            for thunk in slots[idx]:
                                thunk()
                            if idx < nu:
                                produce(units[idx][0], qca, units[idx][1])
                            if 1 <= idx <= nu:
                                expmask(units[idx - 1][0], qca,
                                        units[idx - 1][1])
                            if idx >= 4 and idx - 4 < nu:
                                pv(units[idx - 4][0], qca, units[idx - 4][1],
                                   n_g * 2)
                        epilogues(qca)

                    def qproj_tasks(qc):
                        tasks = []
                        psq = [None] * QH

                        def start_chunk():
                            for d in range(QH):
                                psq[d] = q_pool.tile([128, 512], F32,
                                                     tag=f"q{d}",
                                                     name=f"psq{d}")

                        def ktile(k):
                            def f():
                                if k == 0:
                                    start_chunk()
                                if k % 2 == 0:
                                    ktile.xt4 = load_xt(qc, k // 2)
                                xt_t = ktile.xt4[:, k % 2, :]
                                st, sp = (k == 0), (k == KT_H - 1)
                                for d in range(QH):
                                    nc.tensor.matmul(
                                        psq[d][:, :],
                                        wq_t[k // 4][:, k % 4,
                                                     d * 128:(d + 1) * 128],
                                        xt_t, start=st, stop=sp)
                            return f

                        for k in range(KT_H):
                            tasks.append(ktile(k))
                        for d in range(QH):
                            tasks.append(
                                lambda d=d: rope(psq[d], qt[(d, qc)], qc))
                        return tasks

                    for qc in range(1, NQC):
                        attn_iter(qc - 1, qproj_tasks(qc))

                    # flush: attention chunk 3 + first half of wo projection
                    p3a = [lambda ci=ci, rt=rt, u=u: p3_chain(ci, rt, u, True)
                           for ci, (rt, u) in enumerate(
                               (rt, u) for rt in range(32) for u in range(2))]
                    attn_iter(3, p3a)
                    for ci, (rt, u) in enumerate(
                            (rt, u) for rt in range(32) for u in (2, 3)):
                        p3_chain(ci, rt, u)
    nc.compile()
    return nc


def get_nc():
    if "nc" not in _CACHE:
        _CACHE["nc"] = _build()
    return _CACHE["nc"]


def _bf16(a):
    return np.asarray(a, dtype=mybir.dt.np(mybir.dt.bfloat16))


def prep_in_maps(hidden_states, attention_mask, position_ids, wq, wk, wv, wo):
    hs = np.asarray(hidden_states, dtype=np.float32)
    pos = np.asarray(position_ids)
    wq = np.asarray(wq, dtype=np.float32)
    wk = np.asarray(wk, dtype=np.float32)
    wv = np.asarray(wv, dtype=np.float32)
    wo = np.asarray(wo, dtype=np.float32)

    xt = _bf16(np.ascontiguousarray(hs[0].T))  # [HIDDEN, S]

    inv = 1.0 / (10000.0 ** (np.arange(0, HD, 2, dtype=np.float64) / HD))
    freqs = inv[:, None] * pos[0].astype(np.float64)[None, :]  # [64, S]
    cos = np.cos(freqs).astype(np.float32)
    sin = np.sin(freqs).astype(np.float32)

    perm = np.concatenate([np.arange(0, HD, 2), np.arange(1, HD, 2)])
    scale = np.float32(1.0 / np.sqrt(HD))

    kk = np.arange(128)[:, None]
    qq = np.arange(512)[None, :]
    masks = [_bf16((kk + 128 * o <= qq).astype(np.float32)) for o in range(4)]
    onesc = _bf16(np.ones((128, 1), np.float32))
    onesr = np.ones((1, 128), np.float32)

    in_maps = []
    for i in range(NCORES):
        wq_i = wq[:, i * 512:(i + 1) * 512].reshape(HIDDEN, QH, HD)[:, :, perm]
        wq_i = _bf16(wq_i.reshape(HIDDEN, QH * HD) * scale)
        wk_i = _bf16(wk[:, i * HD:(i + 1) * HD][:, perm])
        wv_i = _bf16(wv[:, i * HD:(i + 1) * HD])
        wo_i = _bf16(wo[i * 512:(i + 1) * 512, :])
        in_maps.append({
            "xt": xt, "wq": wq_i, "wk": wk_i, "wv": wv_i, "wo": wo_i,
            "cosd": cos, "sind": sin,
            "mask0": masks[0], "mask1": masks[1], "mask2": masks[2],
            "mask3": masks[3], "onesc": onesc, "onesr": onesr,
        })
    return in_maps


def kernel(hidden_states, attention_mask, position_ids, wq, wk, wv, wo):
    in_maps = prep_in_maps(hidden_states, attention_mask, position_ids,
                           wq, wk, wv, wo)
    nc = get_nc()
    res = run_bass_kernel_spmd(nc, in_maps, core_ids=list(range(NCORES)))
    total = res.results[0]["out"].astype(np.float32)
    for i in range(1, NCORES):
        total = total + res.results[i]["out"].astype(np.float32)
    return np.ascontiguousarray(total.T).reshape(1, S, HIDDEN)
